# revision 1
# baseline (speedup 1.0000x reference)
"""Trainium2 Bass kernel for nn_Dep_Context_80109730005366.

Math notes (exact restructurings of the reference):
  - ctx = (q @ key) @ value is reassociated as q @ (key @ value); KV is
    [hid, c] so the huge [hw, hw] energy matrix never materializes.
  - The 1x1 conv (proj_W) and the BN scale commute with the bilinear
    upsample, so we contract KV with proj_W into a per-part [hid, hid]
    matrix (KVW) and upsample 10 channels instead of 256.
  - Coord features are input-independent; everything derived from them
    (cf, key/query constant terms) is precomputed on host as tiny matrices.

Sharding: 8 cores = 4 batches x 2 half-part groups. Core k handles batch
n = k//2 and parts {0,1,2} (k even) or {3,4,5} (k odd). Shared per-batch
work (maxpool of p_fea, key, KV) is duplicated across the 2 cores of a
batch; per-part work is split.
"""

import numpy as np

import bass_rust
import concourse.bass as bass
import concourse.tile as tile
from concourse import mybir
from concourse.bass_utils import run_bass_kernel_spmd
from concourse.vector_clock import ScopedClock

EPS = 1e-5
N, C, H, W = 4, 256, 96, 96
HP, WP = 48, 48
HID, PARTS = 10, 6
X = HP * WP  # 2304
PPC = 3          # parts per core
PL = PPC * HID   # planes per core = 30
F32 = mybir.dt.float32

# ---------------------------------------------------------------------------
# Workaround: this container's walrus codegen rejects instructions carrying
# more than a couple of semaphore waits ("Too many sync wait commands").
# TileContext's exit path puts every outstanding wait on one Drain; spread
# them over a chain of single-wait nops instead.
# ---------------------------------------------------------------------------
_MAX_WAITS = 1


def _patched_drain_and_barrier(self, tick_clock, wait_clock):
    nc = self.nc
    drain_inst = nc.sync.drain()
    wait_clock.add_sem_waits(
        drain_inst.ins, ScopedClock({None: tick_clock.global_clock})
    )
    si = drain_inst.ins.sync_info
    waits = list(si.on_wait) if si is not None else []
    updates = list(si.on_update) if si is not None else []
    if len(waits) > _MAX_WAITS:
        drain_inst.ins.sync_info = bass_rust.SyncInfo(
            on_wait=waits[:_MAX_WAITS], on_update=updates
        )
        rest = waits[_MAX_WAITS:]
        for i in range(0, len(rest), _MAX_WAITS):
            nop = nc.sync.nop(nofuse=True, hint="split_drain_wait")
            nop.ins.sync_info = bass_rust.SyncInfo(
                on_wait=rest[i : i + _MAX_WAITS], on_update=[]
            )
    nc.all_engine_barrier()
    assert self.sems is not None
    popped = nc._tile_sem_poison_stack.pop()
    assert popped is self._sem_poison
    nc.clear_and_free_semaphores(list(self.sems.allocated().values()))
    nc.all_engine_barrier()


tile.TileContext._drain_and_barrier = _patched_drain_and_barrier

_BODY_MAX_WAITS = 1


def _split_excess_waits(nc, maxw=_BODY_MAX_WAITS):
    """Post-pass: any instruction carrying more than `maxw` semaphore waits
    gets the excess hoisted onto same-engine nops inserted right before it
    (the engine sequencer blocks on those first, preserving semantics)."""
    eng_map = {
        mybir.EngineType.SP: nc.sync,
        mybir.EngineType.PE: nc.tensor,
        mybir.EngineType.DVE: nc.vector,
        mybir.EngineType.Activation: nc.scalar,
        mybir.EngineType.Pool: nc.gpsimd,
    }

    def make_nop(engine_type, waits):
        bi = eng_map[engine_type].nop(nofuse=True, hint="wait_split")
        # pop it off the tail of the current bb; we'll splice it manually
        cur = nc.cur_bb.bb
        lst = cur.instructions
        assert lst[-1].name == bi.ins.name
        cur.instructions = lst[:-1]
        bi.ins.sync_info = bass_rust.SyncInfo(on_wait=waits, on_update=[])
        return bi.ins

    for bb in nc.m.functions[0].blocks:
        insts = bb.instructions
        out = []
        changed = False
        for inst in insts:
            si = inst.sync_info
            waits = list(si.on_wait) if si is not None else []
            if len(waits) > maxw:
                updates = list(si.on_update) if si is not None else []
                extra, keep = waits[:-maxw], waits[-maxw:]
                for j in range(0, len(extra), maxw):
                    out.append(make_nop(inst.engine, extra[j : j + maxw]))
                inst.sync_info = bass_rust.SyncInfo(on_wait=keep, on_update=updates)
                changed = True
            out.append(inst)
        if changed:
            bb.instructions = out


# ---------------------------------------------------------------------------
# Host-side constant precomputation (all tiny; heavy tensors stay on device)
# ---------------------------------------------------------------------------
def _coord_feats(hp, wp):
    xs = np.arange(wp, dtype=np.float32)
    ys = np.arange(hp, dtype=np.float32)
    xmin = xs / wp * 2 - 1
    xmax = (xs + 1) / wp * 2 - 1
    xctr = (xmin + xmax) / 2
    ymin = ys / hp * 2 - 1
    ymax = (ys + 1) / hp * 2 - 1
    yctr = (ymin + ymax) / 2
    Xb = lambda v: np.broadcast_to(v[None, :], (hp, wp))
    Yb = lambda v: np.broadcast_to(v[:, None], (hp, wp))
    ones = np.ones((hp, wp), np.float32)
    return np.stack(
        [Xb(xmin), Yb(ymin), Xb(xmax), Yb(ymax), Xb(xctr), Yb(yctr),
         ones / wp, ones / hp], axis=0,
    ).astype(np.float32)


def _interp_matrix(out_n, in_n):
    pos = np.arange(out_n, dtype=np.float32) * ((in_n - 1) / (out_n - 1))
    i0 = np.clip(np.floor(pos).astype(np.int64), 0, in_n - 1)
    i1 = np.clip(i0 + 1, 0, in_n - 1)
    w1 = (pos - i0).astype(np.float32)
    M = np.zeros((out_n, in_n), np.float32)
    for r in range(out_n):
        M[r, i0[r]] += 1 - w1[r]
        M[r, i1[r]] += w1[r]
    return M


# ---------------------------------------------------------------------------
# Device program (built once, shared SPMD across all 8 cores)
# ---------------------------------------------------------------------------
def _build_program(reps=1, barrier=False):
    nc = bass.Bass()
    dt = F32

    pfe = nc.dram_tensor("pfe", [C, H * W], dt, kind="ExternalInput")
    hu3 = nc.dram_tensor("hu3", [PL, H * W], dt, kind="ExternalInput")
    # all small constants packed into one [128, 1188] bank (one DMA):
    # cols 0:40 stat0, 40:80 stat1, 80:176 id96, 176:272 mwT, 272:368 mhT,
    # 368:408 id40, 408:438 qstat, 438:468 bnb, 468:1188 kcfT40 (18 blocks of
    # [128, 40]: cols 0:10 = kcfT block, 10:40 zero)
    cbank = nc.dram_tensor("cbank", [128, 1188], dt, kind="ExternalInput")
    qconst = nc.dram_tensor("qconst", [HID, X], dt, kind="ExternalInput")
    out3 = nc.dram_tensor("out3", [PL, H * W], dt, kind="ExternalOutput")

    HCH = 8                # h-chunks per c-tile for p_fea streaming
    RH = H // HCH          # 12 input rows per chunk
    PH = RH // 2           # 6 pooled rows per chunk
    XC = PH * WP           # 288: x-chunk aligned to one pooled h-chunk

    def alt_copy(idx, out, in_):
        # alternate psum->sbuf copies between ACT and DVE to balance engines
        if idx % 2 == 0:
            nc.scalar.copy(out, in_)
        else:
            nc.vector.tensor_copy(out, in_)

    with tile.TileContext(nc) as tc:
      for _rep in range(reps):
        with (
            tc.tile_pool(name="consts", bufs=1) as consts,
            tc.tile_pool(name="pfe_in", bufs=4) as pfe_pool,
            tc.tile_pool(name="p1", bufs=3) as p1_pool,
            tc.tile_pool(name="pf", bufs=1) as pf_pool,
            tc.tile_pool(name="hu", bufs=1) as hu_pool,
            tc.tile_pool(name="big", bufs=1) as big,
            tc.tile_pool(name="small", bufs=2) as small,
            tc.tile_pool(name="psA", bufs=2, space="PSUM") as psA,
            tc.tile_pool(name="psS", bufs=2, space="PSUM") as psS,
            tc.tile_pool(name="psCK", bufs=2, space="PSUM") as psCK,
            tc.tile_pool(name="psU", bufs=2, space="PSUM") as psU,
        ):
            # ---- constants: one packed DMA -------------------------------
            cb = consts.tile([128, 1188], dt, tag="cbank", name="cbank")
            nc.sync.dma_start(cb[:], cbank[:])
            stat_sb = [cb[0:128, 0:40], cb[0:128, 40:80]]
            id96_sb = cb[0:96, 80:176]
            mwT_sb = cb[0:WP, 176:272]
            mhT_sb = cb[0:HP, 272:368]
            id40_sb = cb[0:40, 368:408]
            qstat_sb = cb[0:40, 408:438]
            bnb_sb = cb[0:H, 438:468]
            kcfT40_sb = cb[0:128, 468:1188].rearrange("p (b k) -> p b k", k=40)
            huq = big.tile([40, X], dt, tag="huq")  # 0..29 pooled hu, 30..39 qconst
            nc.gpsimd.dma_start(huq[30:40, :], qconst[:])

            # ---- input DMAs: first chunks, then hu, then the rest --------
            pf_t = [
                pf_pool.tile([128, HP, WP], dt, tag=f"pf{ci}", name=f"pf{ci}")
                for ci in range(2)
            ]
            hu_sb = hu_pool.tile([PL, H * W], dt, tag="hu_sb")

            chunks = []
            for hi in range(HCH):
                for ci in range(2):
                    chunk = pfe_pool.tile([128, RH, W], dt, tag="chunk", name="chunk")
                    nc.sync.dma_start(
                        chunk[:],
                        pfe[ci * 128 : (ci + 1) * 128,
                            hi * RH * W : (hi + 1) * RH * W]
                        .rearrange("c (r w) -> c r w", r=RH),
                    )
                    chunks.append((hi, ci, chunk))
                if hi == 1:
                    nc.sync.dma_start(hu_sb[:], hu3[:])

            # ---- p_fea maxpool (DVE), one chunk at a time ----------------
            for hi, ci, chunk in chunks:
                p1 = p1_pool.tile([128, RH, WP], dt, tag="p1", name="p1")
                ch4 = chunk.rearrange("c r (w2 two) -> c r w2 two", two=2)
                nc.vector.tensor_max(p1[:], ch4[:, :, :, 0], ch4[:, :, :, 1])
                p14 = p1.rearrange("c (h2 two) w -> c h2 two w", two=2)
                nc.vector.tensor_max(
                    pf_t[ci][0:128, hi * PH : (hi + 1) * PH, :],
                    p14[:, :, 0, :],
                    p14[:, :, 1, :],
                )

            # ---- hu maxpool (DVE), 2 h-halves ----------------------------
            h1 = hu_pool.tile([PL, H, WP], dt, tag="h1")
            hu3d = hu_sb.rearrange("p (h w2 two) -> p h w2 two", h=H, two=2)
            h1p = h1.rearrange("p (h2 two) w -> p h2 two w", two=2)
            huq3d = huq[0:PL, :].rearrange("p (h w) -> p h w", h=HP)
            for half in range(2):
                hs = slice(half * (H // 2), (half + 1) * (H // 2))
                nc.vector.tensor_max(
                    h1[:, hs, :], hu3d[:, hs, :, 0], hu3d[:, hs, :, 1]
                )
                hs2 = slice(half * (HP // 2), (half + 1) * (HP // 2))
                nc.vector.tensor_max(
                    huq3d[:, hs2, :], h1p[:, hs2, 0, :], h1p[:, hs2, 1, :]
                )

            # ---- key + WpPf (stat matmul over pf), x-chunks of 288 -------
            keywp_sb = big.tile([40, X], dt, tag="keywp")
            pf_f = [t.rearrange("c h w -> c (h w)") for t in pf_t]
            for xi in range(HCH):
                x0 = xi * XC
                ps = psA.tile([40, XC], dt, tag="psA", name="ps")
                nc.tensor.matmul(
                    ps[:], stat_sb[0], pf_f[0][:, x0 : x0 + XC],
                    start=True, stop=False,
                )
                nc.tensor.matmul(
                    ps[:], stat_sb[1], pf_f[1][:, x0 : x0 + XC],
                    start=False, stop=True,
                )
                alt_copy(xi, keywp_sb[:, x0 : x0 + XC], ps[:])

            # ---- transpose key|WpPf -> [x, 40] blocks; accumulate KVW ----
            kvw_ps = psCK.tile([HID, PL], dt, tag="ck", name="kvw_ps")
            keywpT = big.tile([128, 18, 40], dt, tag="keywpT")
            for b in range(18):
                tp = psS.tile([128, 40], dt, tag="pss", name="tp")
                nc.tensor.transpose(
                    tp[:], keywp_sb[:, b * 128 : (b + 1) * 128], id40_sb
                )
                # copy + fold in the transposed key coord-const (cols 10:40
                # of the kcfT40 block are zero)
                nc.vector.tensor_add(keywpT[:, b, :], tp[:], kcfT40_sb[:, b, :])
            for b in range(18):
                nc.tensor.matmul(
                    kvw_ps[:],
                    keywpT[:, b, 0:HID],
                    keywpT[:, b, HID:40],
                    start=(b == 0),
                    stop=(b == 17),
                )
            kvw_sb = small.tile([HID, PL], dt, tag="kvw_sb")
            nc.vector.tensor_copy(kvw_sb[:], kvw_ps[:])

            # block-diagonal [30, 30] version of KVW (per-part blocks);
            # three partition-shifting SBUF->SBUF DMAs on separate queues
            kvwbd = consts.tile([PL, PL], dt, tag="kvwbd")
            nc.gpsimd.memset(kvwbd[:], 0.0)
            for j, eng in zip(range(PPC), (nc.gpsimd, nc.sync, nc.scalar)):
                eng.dma_start(
                    kvwbd[j * HID : (j + 1) * HID, j * HID : (j + 1) * HID],
                    kvw_sb[:, j * HID : (j + 1) * HID],
                )

            # ---- q_all = qstat.T @ [hu_pool; qconst] ---------------------
            q_sb = big.tile([PL, X], dt, tag="q_sb")
            for xi in range(HCH):
                x0 = xi * XC
                ps = psA.tile([PL, XC], dt, tag="psA", name="ps")
                nc.tensor.matmul(ps[:], qstat_sb, huq[:, x0 : x0 + XC])
                alt_copy(xi + 1, q_sb[:, x0 : x0 + XC], ps[:])

            # ---- ctx, transposed, plane-major free layout ----------------
            # ctxT[w', i*48 + h'] = sum_k q[(j,k), h'w'] KVW_bd[(j,k), i]
            q3 = q_sb.rearrange("p (h w) -> p h w", h=HP)
            ctxT = big.tile([WP, PL * HP], dt, tag="ctxT")
            # view with free dims (h', i): element [w', h', i] = ctxT[w', i*48+h']
            ctxT_hi = ctxT.rearrange("w (i h) -> w h i", i=PL)
            for g in range(3):
                cps = psCK.tile([WP, 16 * PL], dt, tag="ck", name="cps")
                for hh in range(16):
                    hp_i = g * 16 + hh
                    nc.tensor.matmul(
                        cps[:, hh * PL : (hh + 1) * PL],
                        q3[:, hp_i, :],
                        kvwbd[:],
                    )
                # scatter copy psum (h-major) -> ctxT (i-major)
                cps_v = cps.rearrange("w (h i) -> w h i", h=16)
                alt_copy(g, ctxT_hi[:, g * 16 : (g + 1) * 16, :], cps_v[:])

            # ---- upsample stage 1: contract w' (3 big matmuls) -----------
            # a_sb[W, i*48 + h'] = sum_w' Mw[W, w'] ctxT[w', i*48+h']
            a_sb = big.tile([W, PL * HP], dt, tag="a_sb")
            for g, x0 in enumerate(range(0, PL * HP, 512)):
                xn = min(512, PL * HP - x0)
                ups = psU.tile([W, 512], dt, tag="u", name="ups")
                nc.tensor.matmul(ups[:, :xn], mwT_sb, ctxT[:, x0 : x0 + xn])
                alt_copy(g, a_sb[:, x0 : x0 + xn], ups[:, :xn])

            # ---- upsample stage 2 + BN + relu, plane groups of 6 ---------
            out_sb = big.tile([H, PL, W], dt, tag="out_sb")
            out_v = out3.rearrange("i (h w) -> h i w", h=H)
            zeros_sb = small.tile([H, W], dt, tag="zeros", name="zeros", bufs=1)
            nc.gpsimd.memset(zeros_sb[:], 0.0)
            GP = 6
            for gi in range(PL // GP):
                t2w = small.tile([HP, GP * W], dt, tag="t2w", name="t2w")
                for j3 in range(GP // 3):
                    i = gi * GP + 3 * j3
                    t2pool = psS if j3 % 2 == 0 else psCK
                    t2tag = "pss" if j3 % 2 == 0 else "ck"
                    t2 = t2pool.tile([HP, 3 * W], dt, tag=t2tag, name="t2")
                    for q_ in range(3):
                        nc.tensor.transpose(
                            t2[:, q_ * W : (q_ + 1) * W],
                            a_sb[:, (i + q_) * HP : (i + q_ + 1) * HP],
                            id96_sb,
                        )
                    alt_copy(j3, t2w[:, 3 * j3 * W : 3 * (j3 + 1) * W], t2[:])
                uppool = psU if gi % 2 == 0 else psA
                uptag = "u" if gi % 2 == 0 else "psA"
                ups2 = []
                for half in range(2):
                    up = uppool.tile([H, 3 * W], dt, tag=uptag, name="up")
                    nc.tensor.matmul(
                        up[:],
                        mhT_sb,
                        t2w[:, half * 3 * W : (half + 1) * 3 * W],
                    )
                    ups2.append(up)
                for j in range(GP):
                    i = gi * GP + j
                    up = ups2[j // 3]
                    jc = j % 3
                    if j % 2 == 0:
                        nc.scalar.activation(
                            out_sb[:, i, :], up[:, jc * W : (jc + 1) * W],
                            func=mybir.ActivationFunctionType.Relu,
                            bias=bnb_sb[:, i : i + 1],
                            scale=1.0,
                        )
                    else:
                        nc.vector.scalar_tensor_tensor(
                            out_sb[:, i, :], up[:, jc * W : (jc + 1) * W],
                            bnb_sb[:, i : i + 1], zeros_sb[:],
                            op0=mybir.AluOpType.add,
                            op1=mybir.AluOpType.max,
                        )
                nc.sync.dma_start(
                    out_v[:, gi * GP : (gi + 1) * GP, :],
                    out_sb[:, gi * GP : (gi + 1) * GP, :],
                )
        if barrier:
            nc.all_engine_barrier()

    _split_excess_waits(nc)
    return nc


_PROGRAM_CACHE = {}


def _get_program():
    if "nc" not in _PROGRAM_CACHE:
        _PROGRAM_CACHE["nc"] = _build_program()
    return _PROGRAM_CACHE["nc"]


def make_in_maps(p_fea, hu, coord_W, coord_b, query_W, query_b, key_W, key_b,
                 proj_W, bn_gamma, bn_beta, bn_mean, bn_var):
    p_fea = np.asarray(p_fea, np.float32)
    hu = np.asarray(hu, np.float32)

    # ---- host constant folding ------------------------------------------
    cf8 = _coord_feats(HP, WP).reshape(8, X)
    cf = np.asarray(coord_W, np.float32) @ cf8 + np.asarray(coord_b, np.float32)[:, None]
    kcf = np.asarray(key_W, np.float32)[:, C:] @ cf + np.asarray(key_b, np.float32)[:, None]
    qconst = (np.asarray(query_W, np.float32)[:, HID:] @ cf
              + np.asarray(query_b, np.float32)[:, None])
    Mh = _interp_matrix(H, HP)
    Mw = _interp_matrix(W, WP)
    bn_scale = np.asarray(bn_gamma, np.float32) / np.sqrt(np.asarray(bn_var, np.float32) + EPS)
    bn_bias = np.asarray(bn_beta, np.float32) - np.asarray(bn_mean, np.float32) * bn_scale
    WpS = bn_scale[:, :, None] * np.asarray(proj_W, np.float32)  # [parts, hid, c]

    kcfT40 = np.zeros((128, 18, 40), np.float32)
    kcfT40[:, :, 0:HID] = kcf.T.reshape(18, 128, HID).transpose(1, 0, 2)
    kcfT40 = kcfT40.reshape(128, 720)

    qW_huT = np.asarray(query_W, np.float32)[:, :HID].T.copy()  # [10, 10] (in, out)
    keyW_cT = np.asarray(key_W, np.float32)[:, :C].T.copy()     # [256, 10]

    in_maps = []
    for core in range(8):
        n_idx = core // 2
        pset = [0, 1, 2] if core % 2 == 0 else [3, 4, 5]

        statf = np.zeros((C + HID, 40), np.float32)
        statf[:C, 0:HID] = keyW_cT
        statf[C:, 0:HID] = np.eye(HID, dtype=np.float32)
        for j, p in enumerate(pset):
            statf[:C, HID + j * HID : HID + (j + 1) * HID] = WpS[p].T
        stat = statf[0:C]

        qs = np.zeros((40, PL), np.float32)
        for j in range(PPC):
            qs[j * HID : (j + 1) * HID, j * HID : (j + 1) * HID] = qW_huT
            qs[PL : PL + HID, j * HID : (j + 1) * HID] = np.eye(HID, dtype=np.float32)

        bnb = np.zeros((H, PL), np.float32)
        for j, p in enumerate(pset):
            bnb[:, j * HID : (j + 1) * HID] = bn_bias[p][None, :]

        cbank = np.zeros((128, 1188), np.float32)
        cbank[0:128, 0:40] = stat[0:128]
        cbank[0:128, 40:80] = stat[128:256]
        cbank[0:96, 80:176] = np.eye(96, dtype=np.float32)
        cbank[0:WP, 176:272] = Mw.T
        cbank[0:HP, 272:368] = Mh.T
        cbank[0:40, 368:408] = np.eye(40, dtype=np.float32)
        cbank[0:40, 408:438] = qs
        cbank[0:H, 438:468] = bnb
        cbank[0:128, 468:1188] = kcfT40
        in_maps.append({
            "pfe": np.ascontiguousarray(p_fea[n_idx].reshape(C, H * W)),
            "hu3": np.ascontiguousarray(hu[pset, n_idx].reshape(PL, H * W)),
            "cbank": cbank,
            "qconst": np.ascontiguousarray(qconst),
        })
    return in_maps


def assemble_out(results):
    out = np.empty((PARTS, N, HID, H, W), np.float32)
    for core in range(8):
        n_idx = core // 2
        pset = [0, 1, 2] if core % 2 == 0 else [3, 4, 5]
        r = results[core]["out3"].reshape(PPC, HID, H, W)
        out[pset, n_idx] = r
    return out


def kernel(**inputs):
    in_maps = make_in_maps(**inputs)
    nc = _get_program()
    res = run_bass_kernel_spmd(nc, in_maps, core_ids=list(range(8)))
    return assemble_out(res.results)

